# revision 2
# baseline (speedup 1.0000x reference)
"""Trainium2 Bass kernel for nn_CrossMultiheadAttention_44074954391814.

Math (reference):
    q = split_heads(y @ Wq.T + bq); k,v = split_heads(x @ {Wk,Wv}.T + b)
    scores[b,h,i,j] = (q . k)/sqrt(64)           (mask is all-zeros: omitted)
    A[h] = sum_b softmax_j(scores[b,h])          # sum over BATCH
    out[b] = concat_heads(A @ v[b]) @ Wo.T + bo

Sharding: 16 heads / 8 cores = 2 heads per core (128 of 1024 channels).
The batch-sum of attention is per-head, so with head sharding it stays
local to a core - no collective needed.  Each core reads the full x,y
(transposed + fp16 on host) and emits a partial (B*S, D) output (only its
128 channels of the Wo contraction); the host sums the 8 partials + bo.

Per-core dataflow:
  qT,kT (ch=128 part, B*S free) <- W-slice matmuls over 8 k-tiles
  vT    likewise, then DMA-xbar transpose to v (s part, ch free)
  per (i-tile, b, head): scores (i part, j free) on PE (heads row-packed),
    exp+row-sum in one ScalarE pass, 1/r on DVE, P *= 1/r (per-partition),
    A[h] += P via tree adds
  A -> AT via DMA-xbar transpose; AV on PE (heads col-packed, PSUM-acc
  over j-tiles); out-proj on PE; partial out DMA'd as fp16.
"""

import sys

sys.path.insert(0, "/opt/trn_rl_repo")

from contextlib import ExitStack

import numpy as np

import concourse.bass as bass
import concourse.tile as tile
from concourse import bacc, mybir
from concourse.bass import ts
from concourse.bass_utils import run_bass_kernel_spmd

D = 1024          # d_model
HEADS = 16
HD = 64           # head dim
B = 4
S = 1024
BS = B * S        # 4096
NCORES = 8
C = 128           # channels per core (2 heads * 64)
KT = D // 128     # 8 contraction tiles
FP16 = mybir.dt.float16
FP32 = mybir.dt.float32
SCALE = 1.0 / 8.0  # 1/sqrt(HD)


def build_program():
    nc = bacc.Bacc("TRN2", target_bir_lowering=False, debug=False)

    yT = nc.dram_tensor("yT", [D, BS], FP16, kind="ExternalInput").ap()
    xT = nc.dram_tensor("xT", [D, BS], FP16, kind="ExternalInput").ap()
    wqT = nc.dram_tensor("wqT", [D, C], FP16, kind="ExternalInput").ap()
    wkT = nc.dram_tensor("wkT", [D, C], FP16, kind="ExternalInput").ap()
    wvT = nc.dram_tensor("wvT", [D, C], FP16, kind="ExternalInput").ap()
    woT = nc.dram_tensor("woT", [C, D], FP16, kind="ExternalInput").ap()
    bq = nc.dram_tensor("bq", [C, 1], FP32, kind="ExternalInput").ap()
    bk = nc.dram_tensor("bk", [C, 1], FP32, kind="ExternalInput").ap()
    bv = nc.dram_tensor("bv", [C, 1], FP32, kind="ExternalInput").ap()
    out = nc.dram_tensor("out", [BS, D], FP16, kind="ExternalOutput").ap()

    with tile.TileContext(nc) as tc, ExitStack() as ctx:
        consts = ctx.enter_context(tc.tile_pool(name="consts", bufs=1))
        qk = ctx.enter_context(tc.tile_pool(name="qk", bufs=1))
        vpool = ctx.enter_context(tc.tile_pool(name="vpool", bufs=1))

        wq_sb = consts.tile([128, KT, C], FP16, tag="wq")
        wk_sb = consts.tile([128, KT, C], FP16, tag="wk")
        wv_sb = consts.tile([128, KT, C], FP16, tag="wv")
        wo_sb = consts.tile([C, D], FP16, tag="wo")
        bq_sb = consts.tile([C, 1], FP32, tag="bq")
        bk_sb = consts.tile([C, 1], FP32, tag="bk")
        bv_sb = consts.tile([C, 1], FP32, tag="bv")
        for w_sb, w_dram in ((wq_sb, wqT), (wk_sb, wkT), (wv_sb, wvT)):
            nc.sync.dma_start(
                out=w_sb, in_=w_dram.rearrange("(kt p) c -> p kt c", p=128)
            )
        nc.sync.dma_start(out=wo_sb, in_=woT)
        nc.sync.dma_start(out=bq_sb, in_=bq)
        nc.sync.dma_start(out=bk_sb, in_=bk)
        nc.sync.dma_start(out=bv_sb, in_=bv)

        qT = qk.tile([C, BS], FP16, tag="qT")
        kT = qk.tile([C, BS], FP16, tag="kT")
        v16 = vpool.tile([128, BS // 128, C], FP16, tag="v16")

        # ---- Phase 1: QKV projections (chT = W_sliceT.T @ inT per k-tile) ----
        with (
            tc.tile_pool(name="xy", bufs=1) as xy,
            tc.tile_pool(name="vt", bufs=1) as vt,
            tc.tile_pool(name="pp_qkv", bufs=4, space="PSUM") as pp_qkv,
        ):
            y_sb = xy.tile([128, KT, BS], FP16, tag="y")
            x_sb = xy.tile([128, KT, BS], FP16, tag="x")
            for kt in range(KT):
                nc.sync.dma_start(out=y_sb[:, kt, :], in_=yT[ts(kt, 128), :])
                nc.sync.dma_start(out=x_sb[:, kt, :], in_=xT[ts(kt, 128), :])

            vT = vt.tile([C, BS], FP16, tag="vT")
            projs = (
                (wq_sb, y_sb, bq_sb, qT),
                (wk_sb, x_sb, bk_sb, kT),
                (wv_sb, x_sb, bv_sb, vT),
            )
            for w_sb, src, b_sb, dst in projs:
                for n in range(BS // 512):
                    ps = pp_qkv.tile([C, 512], FP32, tag="ps")
                    for kt in range(KT):
                        nc.tensor.matmul(
                            ps,
                            lhsT=w_sb[:, kt, :],
                            rhs=src[:, kt, ts(n, 512)],
                            start=(kt == 0),
                            stop=(kt == KT - 1),
                        )
                    nc.vector.tensor_scalar_add(
                        out=dst[:, ts(n, 512)], in0=ps, scalar1=b_sb
                    )

            # v in (s part, ch free) layout for the AV matmul lhsT
            for jt in range(BS // 128):
                nc.sync.dma_start_transpose(
                    out=v16[:, jt, :], in_=vT[:, ts(jt, 128)]
                )

        # ---- Phase 2: scores + softmax + batch-sum ----
        apool = ctx.enter_context(tc.tile_pool(name="apool", bufs=1))
        A = apool.tile([128, 2, S // 128, S], FP16, tag="A")
        with (
            tc.tile_pool(name="pp_sc", bufs=3, space="PSUM") as pp_sc,
            tc.tile_pool(name="ppool", bufs=10) as ppool,
            tc.tile_pool(name="rpool", bufs=12) as rpool,
            tc.tile_pool(name="tpool", bufs=4) as tpool,
        ):
            for it in range(S // 128):
                Pn = {}
                for b in range(B):
                    for h in range(2):
                        hs = slice(h * 64, (h + 1) * 64)
                        sc = pp_sc.tile([128, S], FP32, tag="sc")
                        for jt in range(2):
                            nc.tensor.matmul(
                                sc[:, ts(jt, 512)],
                                lhsT=qT[hs, b * S + it * 128 : b * S + (it + 1) * 128],
                                rhs=kT[hs, b * S + jt * 512 : b * S + (jt + 1) * 512],
                                start=True,
                                stop=True,
                            )
                        P = ppool.tile([128, S], FP16, tag="P")
                        r = rpool.tile([128, 1], FP32, tag="r")
                        rinv = rpool.tile([128, 1], FP32, tag="rinv")
                        nc.scalar.activation(
                            out=P,
                            in_=sc,
                            func=mybir.ActivationFunctionType.Exp,
                            scale=SCALE,
                            accum_out=r,
                        )
                        nc.vector.reciprocal(out=rinv, in_=r)
                        nc.vector.tensor_scalar_mul(out=P, in0=P, scalar1=rinv)
                        Pn[(b, h)] = P
                for h in range(2):
                    t01 = tpool.tile([128, S], FP16, tag="t01")
                    t23 = tpool.tile([128, S], FP16, tag="t23")
                    nc.vector.tensor_add(t01, Pn[(0, h)], Pn[(1, h)])
                    nc.vector.tensor_add(t23, Pn[(2, h)], Pn[(3, h)])
                    nc.vector.tensor_add(A[:, h, it, :], t01, t23)

        # ---- Phase 3: A -> AT (DMA xbar transpose, 128x128 blocks) ----
        atpool = ctx.enter_context(tc.tile_pool(name="atpool", bufs=1))
        AT = atpool.tile([128, 2, S // 128, S], FP16, tag="AT")
        for h in range(2):
            for jt in range(S // 128):
                for it in range(S // 128):
                    nc.sync.dma_start_transpose(
                        out=AT[:, h, jt, ts(it, 128)],
                        in_=A[:, h, it, ts(jt, 128)],
                    )

        # ---- Phase 4: AV + output projection, per output batch ----
        with (
            tc.tile_pool(name="pp_av", bufs=2, space="PSUM") as pp_av,
            tc.tile_pool(name="pp_o", bufs=2, space="PSUM") as pp_o,
            tc.tile_pool(name="ovpool", bufs=4) as ovpool,
            tc.tile_pool(name="opool", bufs=4) as opool,
        ):
            for b in range(B):
                ov_ps = pp_av.tile([C, S], FP32, tag="ov")
                for h in range(2):
                    hs = slice(h * 64, (h + 1) * 64)
                    for jt in range(S // 128):
                        for n in range(2):
                            nc.tensor.matmul(
                                ov_ps[hs, ts(n, 512)],
                                lhsT=v16[:, b * 8 + jt, hs],
                                rhs=AT[:, h, jt, ts(n, 512)],
                                start=(jt == 0),
                                stop=(jt == S // 128 - 1),
                            )
                ovT = ovpool.tile([C, S], FP16, tag="ovT")
                nc.vector.tensor_copy(ovT, ov_ps)
                for st in range(S // 128):
                    o_ps = pp_o.tile([128, D], FP32, tag="o")
                    for n in range(2):
                        nc.tensor.matmul(
                            o_ps[:, ts(n, 512)],
                            lhsT=ovT[:, ts(st, 128)],
                            rhs=wo_sb[:, ts(n, 512)],
                            start=True,
                            stop=True,
                        )
                    o_sb = opool.tile([128, D], FP16, tag="osb")
                    if st % 2 == 0:
                        nc.vector.tensor_copy(o_sb, o_ps)
                    else:
                        nc.scalar.copy(o_sb, o_ps)
                    nc.sync.dma_start(
                        out=out[b * S + st * 128 : b * S + (st + 1) * 128, :],
                        in_=o_sb,
                    )

    return nc


_PROGRAM = None


def _get_program():
    global _PROGRAM
    if _PROGRAM is None:
        _PROGRAM = build_program()
        _PROGRAM.finalize()
    return _PROGRAM


def kernel(**inputs):
    x = np.asarray(inputs["x"], dtype=np.float32)
    y = np.asarray(inputs["y"], dtype=np.float32)
    Wq = np.asarray(inputs["Wq"], dtype=np.float32)
    Wk = np.asarray(inputs["Wk"], dtype=np.float32)
    Wv = np.asarray(inputs["Wv"], dtype=np.float32)
    Wo = np.asarray(inputs["Wo"], dtype=np.float32)
    bq = np.asarray(inputs["bq"], dtype=np.float32)
    bk = np.asarray(inputs["bk"], dtype=np.float32)
    bv = np.asarray(inputs["bv"], dtype=np.float32)
    bo = np.asarray(inputs["bo"], dtype=np.float32)

    xT16 = np.ascontiguousarray(x.reshape(BS, D).T).astype(np.float16)
    yT16 = np.ascontiguousarray(y.reshape(BS, D).T).astype(np.float16)

    in_maps = []
    for c in range(NCORES):
        rows = slice(c * C, (c + 1) * C)
        in_maps.append(
            {
                "yT": yT16,
                "xT": xT16,
                "wqT": np.ascontiguousarray(Wq[rows, :].T).astype(np.float16),
                "wkT": np.ascontiguousarray(Wk[rows, :].T).astype(np.float16),
                "wvT": np.ascontiguousarray(Wv[rows, :].T).astype(np.float16),
                "woT": np.ascontiguousarray(Wo[:, rows].T).astype(np.float16),
                "bq": bq[rows].reshape(C, 1).astype(np.float32),
                "bk": bk[rows].reshape(C, 1).astype(np.float32),
                "bv": bv[rows].reshape(C, 1).astype(np.float32),
            }
        )

    nc = _get_program()
    res = run_bass_kernel_spmd(nc, in_maps, list(range(NCORES)))

    acc = np.zeros((BS, D), dtype=np.float32)
    for c in range(NCORES):
        acc += res.results[c]["out"].astype(np.float32)
    acc += bo[None, :]
    return acc.reshape(B, S, D)


# revision 11
# speedup vs baseline: 1.6211x; 1.6211x over previous
"""Trainium2 Bass kernel for nn_CrossMultiheadAttention_44074954391814.

Math (reference):
    q = split_heads(y @ Wq.T + bq); k,v = split_heads(x @ {Wk,Wv}.T + b)
    scores[b,h,i,j] = (q . k)/sqrt(64)           (mask is all-zeros: omitted)
    A[h] = sum_b softmax_j(scores[b,h])          # sum over BATCH
    out[b] = concat_heads(A @ v[b]) @ Wo.T + bo

Sharding: 16 heads / 8 cores = 2 heads per core (128 of 1024 channels).
The batch-sum of attention is per-head, so with head sharding it stays
local to a core - no collective needed.  Each core reads the full x,y
(transposed + fp16 on host) and emits a partial (B*S, D) output (only its
128 channels of the Wo contraction); the host sums the 8 partials + bo.

Per-core schedule (phases overlap via Tile dataflow deps):
  warmup matmul stream on PE while the x/y streams DMA in (HAM un-throttle)
  qT,kT,vT (ch=128 part, B*S free): per-batch-quarter matmuls; scores for
    batch b start as soon as q/k quarter b is evacuated
  batch-outer softmax: scores (heads row-packed) -> exp+row-sum (one
    ScalarE pass, accum_out) -> 1/r -> P*(1/r) accumulated into A[h]
  last batch: PE-transpose A row-blocks into AT (psum-bank batched)
  vT -> v via PE transposes; AV (heads col-packed, PSUM-acc over j);
  out-proj; partial out DMA'd as fp16.
"""

import sys

sys.path.insert(0, "/opt/trn_rl_repo")

from contextlib import ExitStack

import numpy as np

import concourse.bass as bass
import concourse.tile as tile
from concourse import bacc, mybir
from concourse.bass import ts
from concourse.bass_utils import run_bass_kernel_spmd
from concourse.masks import make_identity

D = 1024          # d_model
HEADS = 16
HD = 64           # head dim
B = 4
S = 1024
BS = B * S        # 4096
NCORES = 8
C = 128           # channels per core (2 heads * 64)
KT = D // 128     # 8 contraction tiles
FP16 = mybir.dt.float16
FP32 = mybir.dt.float32
SCALE = 1.0 / 8.0  # 1/sqrt(HD)
N_WARMUP = 48


def build_program():
    nc = bacc.Bacc("TRN2", target_bir_lowering=False, debug=False)

    yT = nc.dram_tensor("yT", [D, BS], FP16, kind="ExternalInput").ap()
    xT = nc.dram_tensor("xT", [D, BS], FP16, kind="ExternalInput").ap()
    wqT = nc.dram_tensor("wqT", [D, C], FP16, kind="ExternalInput").ap()
    wkT = nc.dram_tensor("wkT", [D, C], FP16, kind="ExternalInput").ap()
    wvT = nc.dram_tensor("wvT", [D, C], FP16, kind="ExternalInput").ap()
    woT = nc.dram_tensor("woT", [C, D], FP16, kind="ExternalInput").ap()
    bq = nc.dram_tensor("bq", [C, 1], FP32, kind="ExternalInput").ap()
    bk = nc.dram_tensor("bk", [C, 1], FP32, kind="ExternalInput").ap()
    bv = nc.dram_tensor("bv", [C, 1], FP32, kind="ExternalInput").ap()
    out = nc.dram_tensor("out", [BS, D], FP16, kind="ExternalOutput").ap()

    with tile.TileContext(nc) as tc, ExitStack() as ctx:
        consts = ctx.enter_context(tc.tile_pool(name="consts", bufs=1))
        qk = ctx.enter_context(tc.tile_pool(name="qk", bufs=1))
        vpool = ctx.enter_context(tc.tile_pool(name="vpool", bufs=1))

        ident = consts.tile([128, 128], FP16, tag="ident")
        make_identity(nc, ident)

        wq_sb = consts.tile([128, KT, C], FP16, tag="wq")
        wk_sb = consts.tile([128, KT, C], FP16, tag="wk")
        wv_sb = consts.tile([128, KT, C], FP16, tag="wv")
        wo_sb = consts.tile([C, D], FP16, tag="wo")
        bq_sb = consts.tile([C, 1], FP32, tag="bq")
        bk_sb = consts.tile([C, 1], FP32, tag="bk")
        bv_sb = consts.tile([C, 1], FP32, tag="bv")
        for w_sb, w_dram in ((wq_sb, wqT), (wk_sb, wkT), (wv_sb, wvT)):
            nc.sync.dma_start(
                out=w_sb, in_=w_dram.rearrange("(kt p) c -> p kt c", p=128)
            )
        nc.sync.dma_start(out=wo_sb, in_=woT)
        nc.sync.dma_start(out=bq_sb, in_=bq)
        nc.sync.dma_start(out=bk_sb, in_=bk)
        nc.sync.dma_start(out=bv_sb, in_=bv)

        qT = qk.tile([C, BS], FP16, tag="qT")
        kT = qk.tile([C, BS], FP16, tag="kT")
        vT = qk.tile([C, BS], FP16, tag="vT")
        v16 = vpool.tile([128, BS // 128, C], FP16, tag="v16")

        # PE warmup: dummy matmuls with no data deps keep the PE busy while
        # the input streams land, so HAM un-throttles to 2.4 GHz before the
        # first real matmul.  Output psum is never read.
        with (
            tc.tile_pool(name="wup", bufs=1) as wup,
            tc.tile_pool(name="pp_w", bufs=1, space="PSUM") as pp_w,
        ):
            wdummy = wup.tile([128, 512], FP16, tag="wdummy")
            nc.gpsimd.memset(wdummy, 0.0)
            wps = pp_w.tile([128, 512], FP32, tag="wps")
            for _ in range(N_WARMUP):
                nc.tensor.matmul(
                    wps, lhsT=wdummy[:, 0:128], rhs=wdummy, start=True, stop=True
                )

        apool = ctx.enter_context(tc.tile_pool(name="apool", bufs=1))
        atpool = ctx.enter_context(tc.tile_pool(name="atpool", bufs=1))
        A = apool.tile([128, 2, S // 128, S], FP16, tag="A")
        AT = atpool.tile([128, 2, S // 128, S], FP16, tag="AT")

        with (
            tc.tile_pool(name="xy", bufs=2) as xy,
            tc.tile_pool(name="pp_qkv", bufs=2, space="PSUM") as pp_qkv,
            tc.tile_pool(name="tp", bufs=2, space="PSUM") as tp,
            tc.tile_pool(name="pp_sc", bufs=2, space="PSUM") as pp_sc,
            tc.tile_pool(name="ppool", bufs=4) as ppool,
            tc.tile_pool(name="pnpool", bufs=4) as pnpool,
            tc.tile_pool(name="rpool", bufs=12) as rpool,
        ):
            # ---- QKV projections, streamed by batch-sized column quarters --
            def proj_quarter(src_q, w_sb, b_sb, dst, g):
                for n2 in range(2):
                    n = g * 2 + n2
                    ps = pp_qkv.tile([C, 512], FP32, tag="ps")
                    for kt in range(KT):
                        nc.tensor.matmul(
                            ps,
                            lhsT=w_sb[:, kt, :],
                            rhs=src_q[:, kt, ts(n2, 512)],
                            start=(kt == 0),
                            stop=(kt == KT - 1),
                        )
                    nc.vector.tensor_scalar_add(
                        out=dst[:, ts(n, 512)], in0=ps, scalar1=b_sb
                    )

            for g in range(4):
                yq = xy.tile([128, KT, 1024], FP16, tag="yq")
                nc.sync.dma_start(
                    out=yq,
                    in_=yT[:, ts(g, 1024)].rearrange("(kt p) s -> p kt s", p=128),
                )
                proj_quarter(yq, wq_sb, bq_sb, qT, g)
            for g in range(4):
                xq = xy.tile([128, KT, 1024], FP16, tag="xq")
                nc.sync.dma_start(
                    out=xq,
                    in_=xT[:, ts(g, 1024)].rearrange("(kt p) s -> p kt s", p=128),
                )
                proj_quarter(xq, wk_sb, bk_sb, kT, g)
                proj_quarter(xq, wv_sb, bv_sb, vT, g)

            # ---- softmax, batch-outer so batch b starts once quarter b done
            for b in range(B):
                for it in range(S // 128):
                    for h in range(2):
                        hs = slice(h * 64, (h + 1) * 64)
                        sc = pp_sc.tile([128, S], FP32, tag="sc")
                        for jt in range(2):
                            nc.tensor.matmul(
                                sc[:, ts(jt, 512)],
                                lhsT=qT[
                                    hs, b * S + it * 128 : b * S + (it + 1) * 128
                                ],
                                rhs=kT[
                                    hs, b * S + jt * 512 : b * S + (jt + 1) * 512
                                ],
                                start=True,
                                stop=True,
                            )
                        P = ppool.tile([128, S], FP16, tag="P")
                        r = rpool.tile([128, 1], FP32, tag="r")
                        rinv = rpool.tile([128, 1], FP32, tag="rinv")
                        nc.scalar.activation(
                            out=P,
                            in_=sc,
                            func=mybir.ActivationFunctionType.Exp,
                            scale=SCALE,
                            accum_out=r,
                        )
                        nc.vector.reciprocal(out=rinv, in_=r)
                        if b == 0:
                            nc.vector.tensor_scalar_mul(
                                out=A[:, h, it, :], in0=P, scalar1=rinv
                            )
                        else:
                            Pw = pnpool.tile([128, S], FP16, tag="Pn")
                            nc.vector.tensor_scalar_mul(out=Pw, in0=P, scalar1=rinv)
                            nc.vector.tensor_add(
                                A[:, h, it, :], A[:, h, it, :], Pw
                            )
                        if b == B - 1:
                            # A row-block final: transpose into AT
                            aps = tp.tile([128, 1024], FP16, tag="tp")
                            for jt in range(8):
                                nc.tensor.matmul(
                                    aps[:, ts(jt, 128)],
                                    lhsT=A[:, h, it, ts(jt, 128)],
                                    rhs=ident,
                                    is_transpose=True,
                                    start=(jt == 0),
                                    stop=(jt == 7),
                                )
                            nc.vector.tensor_copy(AT[:, h, :, ts(it, 128)], aps)

            # ---- v to (s part, ch free) via PE transposes ----
            for g in range(4):
                vps = tp.tile([128, 1024], FP16, tag="tp")
                for k in range(8):
                    jt = g * 8 + k
                    nc.tensor.matmul(
                        vps[:, ts(k, 128)],
                        lhsT=vT[:, ts(jt, 128)],
                        rhs=ident,
                        is_transpose=True,
                        start=(k == 0),
                        stop=(k == 7),
                    )
                nc.vector.tensor_copy(v16[:, g * 8 : (g + 1) * 8, :], vps)

        # ---- AV + output projection, per output batch ----
        with (
            tc.tile_pool(name="pp_av", bufs=2, space="PSUM") as pp_av,
            tc.tile_pool(name="pp_o", bufs=2, space="PSUM") as pp_o,
            tc.tile_pool(name="ovpool", bufs=4) as ovpool,
            tc.tile_pool(name="opool", bufs=4) as opool,
        ):
            for b in range(B):
                ov_ps = pp_av.tile([C, S], FP32, tag="ov")
                for h in range(2):
                    hs = slice(h * 64, (h + 1) * 64)
                    for jt in range(S // 128):
                        for n in range(2):
                            nc.tensor.matmul(
                                ov_ps[hs, ts(n, 512)],
                                lhsT=v16[:, b * 8 + jt, hs],
                                rhs=AT[:, h, jt, ts(n, 512)],
                                start=(jt == 0),
                                stop=(jt == S // 128 - 1),
                            )
                ovT = ovpool.tile([C, S], FP16, tag="ovT")
                nc.vector.tensor_copy(ovT, ov_ps)
                for st in range(S // 128):
                    o_ps = pp_o.tile([128, D], FP32, tag="o")
                    for n in range(2):
                        nc.tensor.matmul(
                            o_ps[:, ts(n, 512)],
                            lhsT=ovT[:, ts(st, 128)],
                            rhs=wo_sb[:, ts(n, 512)],
                            start=True,
                            stop=True,
                        )
                    o_sb = opool.tile([128, D], FP16, tag="osb")
                    if st % 2 == 0:
                        nc.vector.tensor_copy(o_sb, o_ps)
                    else:
                        nc.scalar.copy(o_sb, o_ps)
                    nc.sync.dma_start(
                        out=out[b * S + st * 128 : b * S + (st + 1) * 128, :],
                        in_=o_sb,
                    )

    return nc


_PROGRAM = None


def _get_program():
    global _PROGRAM
    if _PROGRAM is None:
        _PROGRAM = build_program()
        _PROGRAM.finalize()
    return _PROGRAM


def kernel(**inputs):
    x = np.asarray(inputs["x"], dtype=np.float32)
    y = np.asarray(inputs["y"], dtype=np.float32)
    Wq = np.asarray(inputs["Wq"], dtype=np.float32)
    Wk = np.asarray(inputs["Wk"], dtype=np.float32)
    Wv = np.asarray(inputs["Wv"], dtype=np.float32)
    Wo = np.asarray(inputs["Wo"], dtype=np.float32)
    bq = np.asarray(inputs["bq"], dtype=np.float32)
    bk = np.asarray(inputs["bk"], dtype=np.float32)
    bv = np.asarray(inputs["bv"], dtype=np.float32)
    bo = np.asarray(inputs["bo"], dtype=np.float32)

    xT16 = np.ascontiguousarray(x.reshape(BS, D).T).astype(np.float16)
    yT16 = np.ascontiguousarray(y.reshape(BS, D).T).astype(np.float16)

    in_maps = []
    for c in range(NCORES):
        rows = slice(c * C, (c + 1) * C)
        in_maps.append(
            {
                "yT": yT16,
                "xT": xT16,
                "wqT": np.ascontiguousarray(Wq[rows, :].T).astype(np.float16),
                "wkT": np.ascontiguousarray(Wk[rows, :].T).astype(np.float16),
                "wvT": np.ascontiguousarray(Wv[rows, :].T).astype(np.float16),
                "woT": np.ascontiguousarray(Wo[:, rows].T).astype(np.float16),
                "bq": bq[rows].reshape(C, 1).astype(np.float32),
                "bk": bk[rows].reshape(C, 1).astype(np.float32),
                "bv": bv[rows].reshape(C, 1).astype(np.float32),
            }
        )

    nc = _get_program()
    res = run_bass_kernel_spmd(nc, in_maps, list(range(NCORES)))

    acc = np.zeros((BS, D), dtype=np.float32)
    for c in range(NCORES):
        acc += res.results[c]["out"].astype(np.float32)
    acc += bo[None, :]
    return acc.reshape(B, S, D)


# revision 12
# speedup vs baseline: 1.8449x; 1.1381x over previous
"""Trainium2 Bass kernel for nn_CrossMultiheadAttention_44074954391814.

Math (reference):
    q = split_heads(y @ Wq.T + bq); k,v = split_heads(x @ {Wk,Wv}.T + b)
    scores[b,h,i,j] = (q . k)/sqrt(64)           (mask is all-zeros: omitted)
    A[h] = sum_b softmax_j(scores[b,h])          # sum over BATCH
    out[b] = concat_heads(A @ v[b]) @ Wo.T + bo

Sharding: 16 heads / 8 cores = 2 heads per core (128 of 1024 channels).
The batch-sum of attention is per-head, so with head sharding it stays
local to a core - no collective needed.  Each core reads the full x,y
(transposed + fp16 on host) and emits a partial (B*S, D) output (only its
128 channels of the Wo contraction); the host sums the 8 partials + bo.

Per-core schedule (phases overlap via Tile dataflow deps):
  warmup matmul stream on PE while the x/y streams DMA in (HAM un-throttle)
  qT,kT,vT (ch=128 part, B*S free): per-batch-quarter matmuls; scores for
    batch b start as soon as q/k quarter b is evacuated
  batch-outer softmax: scores (heads row-packed) -> exp+row-sum (one
    ScalarE pass, accum_out) -> 1/r -> P*(1/r) accumulated into A[h]
  last batch: PE-transpose A row-blocks into AT (psum-bank batched)
  vT -> v via PE transposes; AV (heads col-packed, PSUM-acc over j);
  out-proj; partial out DMA'd as fp16.
"""

import sys

sys.path.insert(0, "/opt/trn_rl_repo")

from contextlib import ExitStack

import numpy as np

import concourse.bass as bass
import concourse.tile as tile
from concourse import bacc, mybir
from concourse.bass import ts
from concourse.bass_utils import run_bass_kernel_spmd
from concourse.masks import make_identity

D = 1024          # d_model
HEADS = 16
HD = 64           # head dim
B = 4
S = 1024
BS = B * S        # 4096
NCORES = 8
C = 128           # channels per core (2 heads * 64)
KT = D // 128     # 8 contraction tiles
FP16 = mybir.dt.float16
FP32 = mybir.dt.float32
SCALE = 1.0 / 8.0  # 1/sqrt(HD)
N_WARMUP = 48


def build_program():
    nc = bacc.Bacc("TRN2", target_bir_lowering=False, debug=False)

    yT = nc.dram_tensor("yT", [D, BS], FP16, kind="ExternalInput").ap()
    xT = nc.dram_tensor("xT", [D, BS], FP16, kind="ExternalInput").ap()
    wqT = nc.dram_tensor("wqT", [D, C], FP16, kind="ExternalInput").ap()
    wkT = nc.dram_tensor("wkT", [D, C], FP16, kind="ExternalInput").ap()
    wvT = nc.dram_tensor("wvT", [D, C], FP16, kind="ExternalInput").ap()
    woT = nc.dram_tensor("woT", [C, D], FP16, kind="ExternalInput").ap()
    bq = nc.dram_tensor("bq", [C, 1], FP32, kind="ExternalInput").ap()
    bk = nc.dram_tensor("bk", [C, 1], FP32, kind="ExternalInput").ap()
    bv = nc.dram_tensor("bv", [C, 1], FP32, kind="ExternalInput").ap()
    out = nc.dram_tensor("out", [BS, D], FP16, kind="ExternalOutput").ap()

    with tile.TileContext(nc) as tc, ExitStack() as ctx:
        consts = ctx.enter_context(tc.tile_pool(name="consts", bufs=1))
        qk = ctx.enter_context(tc.tile_pool(name="qk", bufs=1))
        vpool = ctx.enter_context(tc.tile_pool(name="vpool", bufs=1))

        ident = consts.tile([128, 128], FP16, tag="ident")
        make_identity(nc, ident)

        wq_sb = consts.tile([128, KT, C], FP16, tag="wq")
        wk_sb = consts.tile([128, KT, C], FP16, tag="wk")
        wv_sb = consts.tile([128, KT, C], FP16, tag="wv")
        wo_sb = consts.tile([C, D], FP16, tag="wo")
        bq_sb = consts.tile([C, 1], FP32, tag="bq")
        bk_sb = consts.tile([C, 1], FP32, tag="bk")
        bv_sb = consts.tile([C, 1], FP32, tag="bv")
        for w_sb, w_dram in ((wq_sb, wqT), (wk_sb, wkT), (wv_sb, wvT)):
            nc.sync.dma_start(
                out=w_sb, in_=w_dram.rearrange("(kt p) c -> p kt c", p=128)
            )
        nc.sync.dma_start(out=wo_sb, in_=woT)
        nc.sync.dma_start(out=bq_sb, in_=bq)
        nc.sync.dma_start(out=bk_sb, in_=bk)
        nc.sync.dma_start(out=bv_sb, in_=bv)

        qT = qk.tile([C, BS], FP16, tag="qT")
        kT = qk.tile([C, BS], FP16, tag="kT")
        vT = qk.tile([C, BS], FP16, tag="vT")
        v16 = vpool.tile([128, BS // 128, C], FP16, tag="v16")

        # PE warmup: dummy matmuls with no data deps keep the PE busy while
        # the input streams land, so HAM un-throttles to 2.4 GHz before the
        # first real matmul.  Output psum is never read.
        with (
            tc.tile_pool(name="wup", bufs=1) as wup,
            tc.tile_pool(name="pp_w", bufs=1, space="PSUM") as pp_w,
        ):
            wdummy = wup.tile([128, 512], FP16, tag="wdummy")
            nc.gpsimd.memset(wdummy, 0.0)
            wps = pp_w.tile([128, 512], FP32, tag="wps")
            for _ in range(N_WARMUP):
                nc.tensor.matmul(
                    wps, lhsT=wdummy[:, 0:128], rhs=wdummy, start=True, stop=True
                )

        apool = ctx.enter_context(tc.tile_pool(name="apool", bufs=1))
        atpool = ctx.enter_context(tc.tile_pool(name="atpool", bufs=1))
        A = apool.tile([128, 2, S // 128, S], FP16, tag="A")
        AT = atpool.tile([128, 2, S // 128, S], FP16, tag="AT")

        with (
            tc.tile_pool(name="xy", bufs=3) as xy,
            tc.tile_pool(name="pp_qkv", bufs=2, space="PSUM") as pp_qkv,
            tc.tile_pool(name="tp", bufs=2, space="PSUM") as tp,
            tc.tile_pool(name="pp_sc", bufs=2, space="PSUM") as pp_sc,
            tc.tile_pool(name="ppool", bufs=4) as ppool,
            tc.tile_pool(name="pnpool", bufs=4) as pnpool,
            tc.tile_pool(name="rpool", bufs=12) as rpool,
        ):
            # ---- QKV projections, streamed by batch-sized column quarters --
            def proj_quarter(src_q, w_sb, b_sb, dst, g):
                for n2 in range(2):
                    n = g * 2 + n2
                    ps = pp_qkv.tile([C, 512], FP32, tag="ps")
                    for kt in range(KT):
                        nc.tensor.matmul(
                            ps,
                            lhsT=w_sb[:, kt, :],
                            rhs=src_q[:, kt, ts(n2, 512)],
                            start=(kt == 0),
                            stop=(kt == KT - 1),
                        )
                    nc.vector.tensor_scalar_add(
                        out=dst[:, ts(n, 512)], in0=ps, scalar1=b_sb
                    )

            def load_quarter(src_dram, g, tag):
                q = xy.tile([128, KT, 1024], FP16, tag=tag)
                nc.sync.dma_start(
                    out=q,
                    in_=src_dram[:, ts(g, 1024)].rearrange(
                        "(kt p) s -> p kt s", p=128
                    ),
                )
                return q

            # interleave emission so batch b's softmax sits right behind
            # quarter b's q/k projections in each engine queue
            yq = load_quarter(yT, 0, "xyq")
            xq = load_quarter(xT, 0, "xyq")
            proj_quarter(yq, wq_sb, bq_sb, qT, 0)
            proj_quarter(xq, wk_sb, bk_sb, kT, 0)
            for b in range(B):
                if b < B - 1:
                    yq2 = load_quarter(yT, b + 1, "xyq")
                    xq2 = load_quarter(xT, b + 1, "xyq")
                    proj_quarter(yq2, wq_sb, bq_sb, qT, b + 1)
                    proj_quarter(xq2, wk_sb, bk_sb, kT, b + 1)
                proj_quarter(xq, wv_sb, bv_sb, vT, b)
                xq = xq2 if b < B - 1 else None
                for it in range(S // 128):
                    for h in range(2):
                        hs = slice(h * 64, (h + 1) * 64)
                        sc = pp_sc.tile([128, S], FP32, tag="sc")
                        for jt in range(2):
                            nc.tensor.matmul(
                                sc[:, ts(jt, 512)],
                                lhsT=qT[
                                    hs, b * S + it * 128 : b * S + (it + 1) * 128
                                ],
                                rhs=kT[
                                    hs, b * S + jt * 512 : b * S + (jt + 1) * 512
                                ],
                                start=True,
                                stop=True,
                            )
                        P = ppool.tile([128, S], FP16, tag="P")
                        r = rpool.tile([128, 1], FP32, tag="r")
                        rinv = rpool.tile([128, 1], FP32, tag="rinv")
                        nc.scalar.activation(
                            out=P,
                            in_=sc,
                            func=mybir.ActivationFunctionType.Exp,
                            scale=SCALE,
                            accum_out=r,
                        )
                        nc.vector.reciprocal(out=rinv, in_=r)
                        if b == 0:
                            nc.vector.tensor_scalar_mul(
                                out=A[:, h, it, :], in0=P, scalar1=rinv
                            )
                        else:
                            Pw = pnpool.tile([128, S], FP16, tag="Pn")
                            nc.vector.tensor_scalar_mul(out=Pw, in0=P, scalar1=rinv)
                            nc.vector.tensor_add(
                                A[:, h, it, :], A[:, h, it, :], Pw
                            )
                        if b == B - 1:
                            # A row-block final: transpose into AT
                            aps = tp.tile([128, 1024], FP16, tag="tp")
                            for jt in range(8):
                                nc.tensor.matmul(
                                    aps[:, ts(jt, 128)],
                                    lhsT=A[:, h, it, ts(jt, 128)],
                                    rhs=ident,
                                    is_transpose=True,
                                    start=(jt == 0),
                                    stop=(jt == 7),
                                )
                            nc.vector.tensor_copy(AT[:, h, :, ts(it, 128)], aps)

            # ---- v to (s part, ch free) via PE transposes ----
            for g in range(4):
                vps = tp.tile([128, 1024], FP16, tag="tp")
                for k in range(8):
                    jt = g * 8 + k
                    nc.tensor.matmul(
                        vps[:, ts(k, 128)],
                        lhsT=vT[:, ts(jt, 128)],
                        rhs=ident,
                        is_transpose=True,
                        start=(k == 0),
                        stop=(k == 7),
                    )
                nc.vector.tensor_copy(v16[:, g * 8 : (g + 1) * 8, :], vps)

        # ---- AV + output projection, per output batch ----
        with (
            tc.tile_pool(name="pp_av", bufs=2, space="PSUM") as pp_av,
            tc.tile_pool(name="pp_o", bufs=2, space="PSUM") as pp_o,
            tc.tile_pool(name="ovpool", bufs=4) as ovpool,
            tc.tile_pool(name="opool", bufs=4) as opool,
        ):
            for b in range(B):
                ov_ps = pp_av.tile([C, S], FP32, tag="ov")
                for h in range(2):
                    hs = slice(h * 64, (h + 1) * 64)
                    for jt in range(S // 128):
                        for n in range(2):
                            nc.tensor.matmul(
                                ov_ps[hs, ts(n, 512)],
                                lhsT=v16[:, b * 8 + jt, hs],
                                rhs=AT[:, h, jt, ts(n, 512)],
                                start=(jt == 0),
                                stop=(jt == S // 128 - 1),
                            )
                ovT = ovpool.tile([C, S], FP16, tag="ovT")
                nc.vector.tensor_copy(ovT, ov_ps)
                for st in range(S // 128):
                    o_ps = pp_o.tile([128, D], FP32, tag="o")
                    for n in range(2):
                        nc.tensor.matmul(
                            o_ps[:, ts(n, 512)],
                            lhsT=ovT[:, ts(st, 128)],
                            rhs=wo_sb[:, ts(n, 512)],
                            start=True,
                            stop=True,
                        )
                    o_sb = opool.tile([128, D], FP16, tag="osb")
                    if st % 2 == 0:
                        nc.vector.tensor_copy(o_sb, o_ps)
                    else:
                        nc.scalar.copy(o_sb, o_ps)
                    nc.sync.dma_start(
                        out=out[b * S + st * 128 : b * S + (st + 1) * 128, :],
                        in_=o_sb,
                    )

    return nc


_PROGRAM = None


def _get_program():
    global _PROGRAM
    if _PROGRAM is None:
        _PROGRAM = build_program()
        _PROGRAM.finalize()
    return _PROGRAM


def kernel(**inputs):
    x = np.asarray(inputs["x"], dtype=np.float32)
    y = np.asarray(inputs["y"], dtype=np.float32)
    Wq = np.asarray(inputs["Wq"], dtype=np.float32)
    Wk = np.asarray(inputs["Wk"], dtype=np.float32)
    Wv = np.asarray(inputs["Wv"], dtype=np.float32)
    Wo = np.asarray(inputs["Wo"], dtype=np.float32)
    bq = np.asarray(inputs["bq"], dtype=np.float32)
    bk = np.asarray(inputs["bk"], dtype=np.float32)
    bv = np.asarray(inputs["bv"], dtype=np.float32)
    bo = np.asarray(inputs["bo"], dtype=np.float32)

    xT16 = np.ascontiguousarray(x.reshape(BS, D).T).astype(np.float16)
    yT16 = np.ascontiguousarray(y.reshape(BS, D).T).astype(np.float16)

    in_maps = []
    for c in range(NCORES):
        rows = slice(c * C, (c + 1) * C)
        in_maps.append(
            {
                "yT": yT16,
                "xT": xT16,
                "wqT": np.ascontiguousarray(Wq[rows, :].T).astype(np.float16),
                "wkT": np.ascontiguousarray(Wk[rows, :].T).astype(np.float16),
                "wvT": np.ascontiguousarray(Wv[rows, :].T).astype(np.float16),
                "woT": np.ascontiguousarray(Wo[:, rows].T).astype(np.float16),
                "bq": bq[rows].reshape(C, 1).astype(np.float32),
                "bk": bk[rows].reshape(C, 1).astype(np.float32),
                "bv": bv[rows].reshape(C, 1).astype(np.float32),
            }
        )

    nc = _get_program()
    res = run_bass_kernel_spmd(nc, in_maps, list(range(NCORES)))

    acc = np.zeros((BS, D), dtype=np.float32)
    for c in range(NCORES):
        acc += res.results[c]["out"].astype(np.float32)
    acc += bo[None, :]
    return acc.reshape(B, S, D)
